# revision 4
# baseline (speedup 1.0000x reference)
"""Trainium2 Bass kernel for nn_AnteLayer (fuzzy-rule antecedents over graph edges).

Per edge e: x1 = feat[dst,0]-feat[src,0], x2 = feat[dst,1]-feat[src,1],
ante[e, 3j+k] = exp(-2*(x1-c_j)^2) * exp(-2*(x2-c_k)^2),  c in {-1, 0, 1}.

Distribution: edge-parallel across 8 NeuronCores (800K edges each). The host
stages per-edge endpoint features (xy_src / xy_dst); each core runs a fully
pipelined streaming kernel: DMA-in -> subtract (GPSIMD) -> 3x Derivative_Erf
gaussians (ACT) -> 9 rule products (DVE scalar_tensor_tensor) -> DMA-out.

exp(-2(x-c)^2) == (sqrt(pi)/2) * Derivative_Erf(sqrt(2)*x - sqrt(2)*c), so one
ACT op per membership center; the pi/4 factor folds into the product stage.
"""
import sys

for _p in ("/opt/trn_rl_repo", "/opt/pypackages"):
    if _p not in sys.path:
        sys.path.insert(0, _p)

import math
import numpy as np

import concourse.bass as bass
import concourse.mybir as mybir
from concourse import bacc, tile
from concourse.bass_utils import run_bass_kernel_spmd

N_CORES = 8
N_EDGES = 6400000
P = 128                       # SBUF partitions
E_CORE = N_EDGES // N_CORES   # 800000 edges per core
R = E_CORE // P               # 6250 edges per partition
T = 625                       # edges per partition per tile
NT = R // T                   # tiles per core

MF_CENTERS = (-1.0, 0.0, 1.0)
SQRT2 = math.sqrt(2.0)
PI_4 = math.pi / 4.0

_nc_cache = {}


def _build():
    if "nc" in _nc_cache:
        return _nc_cache["nc"]
    nc = bacc.Bacc("TRN2", target_bir_lowering=False)
    f32 = mybir.dt.float32
    xy_s_ext = nc.declare_dram_parameter("xy_src", [P, R, 2], f32, isOutput=False)
    xy_d_ext = nc.declare_dram_parameter("xy_dst", [P, R, 2], f32, isOutput=False)
    out_ext = nc.declare_dram_parameter("out", [P, R, 9], f32, isOutput=True)

    with tile.TileContext(nc) as tc:
        with (
            tc.tile_pool(name="consts", bufs=1) as consts,
            tc.tile_pool(name="xin", bufs=3) as xin,
            tc.tile_pool(name="mid", bufs=2) as mid,
            tc.tile_pool(name="oute", bufs=3) as oute,
        ):
            bias_aps = []
            for ci, c in enumerate(MF_CENTERS):
                b = consts.tile([P, 1], f32, tag=f"bias{ci}")
                nc.vector.memset(b[:, :], -SQRT2 * c)
                bias_aps.append(b)
            for it in range(NT):
                sl = slice(it * T, (it + 1) * T)
                xy_s = xin.tile([P, T, 2], f32, tag="xy_s")
                xy_d = xin.tile([P, T, 2], f32, tag="xy_d")
                nc.sync.dma_start(out=xy_s[:, :, :], in_=xy_s_ext[:, sl, :])
                nc.sync.dma_start(out=xy_d[:, :, :], in_=xy_d_ext[:, sl, :])

                # X[p,t,0] = x1, X[p,t,1] = x2 -- one [P, 2T] subtract on GPSIMD
                x = mid.tile([P, T, 2], f32, tag="x")
                nc.gpsimd.tensor_sub(x[:, :, :], xy_d[:, :, :], xy_s[:, :, :])

                # D[p,c,t,m] = Derivative_Erf(sqrt2*X - sqrt2*center_c)
                d = mid.tile([P, 3, T, 2], f32, tag="d")
                for ci, c in enumerate(MF_CENTERS):
                    nc.scalar.activation(
                        d[:, ci, :, :],
                        x[:, :, :],
                        mybir.ActivationFunctionType.Derivative_Erf,
                        bias=bias_aps[ci][:, :],
                        scale=SQRT2,
                    )

                # ante[p,t,3j+k] = (pi/4) * D[p,j,t,0] * D[p,k,t,1]
                ante = oute.tile([P, T, 9], f32, tag="ante")
                dy = d[:, :, :, 1]                      # [P, 3, T] stride-2
                for j in range(3):
                    dxj_base = d[:, j, :, 0]
                    ap_list = list(dxj_base.ap)
                    dxj = bass.AP(
                        dxj_base.tensor, dxj_base.offset,
                        [list(ap_list[0]), [0, 3], list(ap_list[1])],
                    )
                    outj = ante[:, :, 3 * j:3 * j + 3].rearrange("p t k -> p k t")
                    nc.vector.scalar_tensor_tensor(
                        outj, dxj, PI_4, dy,
                        op0=mybir.AluOpType.mult,
                        op1=mybir.AluOpType.mult,
                    )

                nc.sync.dma_start(out=out_ext[:, sl, :], in_=ante[:, :, :])

    nc.compile()
    _nc_cache["nc"] = nc
    return nc


def kernel(feat, edge_src, edge_dst, etypes):
    feat = np.asarray(feat, dtype=np.float32)
    edge_src = np.asarray(edge_src, dtype=np.int32)
    edge_dst = np.asarray(edge_dst, dtype=np.int32)
    del etypes  # unused by the reference computation

    nc = _build()

    feat2 = np.ascontiguousarray(feat[:, :2])  # only coords participate
    in_maps = []
    for c in range(N_CORES):
        sl = slice(c * E_CORE, (c + 1) * E_CORE)
        xy_s = feat2[edge_src[sl]].reshape(P, R, 2)
        xy_d = feat2[edge_dst[sl]].reshape(P, R, 2)
        in_maps.append({"xy_src": xy_s, "xy_dst": xy_d})

    res = run_bass_kernel_spmd(nc, in_maps, core_ids=list(range(N_CORES)))
    out = np.empty((N_EDGES, 9), dtype=np.float32)
    for c in range(N_CORES):
        out[c * E_CORE:(c + 1) * E_CORE] = res.results[c]["out"].reshape(E_CORE, 9)
    return out
